# revision 5
# baseline (speedup 1.0000x reference)
"""CoulombLayer Trainium2 kernel (8 NeuronCores, SPMD via bass).

Sharding strategy (host-side prep inside kernel(), device does the math):
  * Atoms are sharded contiguously across the 8 cores (62500 atoms/core) and
    edges are sharded by their TARGET atom (edge_index[0]) — a 1D vertex-cut
    graph partition. Every edge of an atom lands on that atom's core, so the
    per-atom segment sum is core-local and no collective is needed.
  * Within a core, edges are laid out CSR-style: each atom owns K fixed
    slots (K = max in-degree over the dataset, padded with qj=0 so padding
    contributes exactly 0). The device kernel then computes, for all 16M
    edge slots: chi(d) (PhysNet smooth-damped 1/r), term = qj_c * chi, a
    per-atom reduction over the K slots, and the final qi_c[a]/2 scaling.
  * The charge-neutrality correction (a 500k->5k segment sum) and the
    per-edge gather of the corrected source charge qj_c = qi_c[edge_index[1]]
    are index-driven data-layout steps done on host as part of building the
    shard layout (this walrus/ucode combination has no usable scalar-gather
    primitive; all FLOP-bearing work per edge slot runs on device).

Device pipeline per core (125 SBUF partitions x 500 atoms x K slots):
  stream (d, qj) tiles -> DVE/ACT chi math -> per-atom K-slot reduce ->
  scale by qi_c/2 -> store 62500 energies. Output is the concat of cores.
"""

import json as _json
import numpy as np

N_CORES = 8
N_ATOMS = 500_000
N_MOL = 5_000
N_EDGES = 16_000_000
CUTOFF = 10.0
P = 125                 # SBUF partitions used (125 * 500 = 62500 atoms/core)
APP = 500               # atoms per partition
CAT = 25                # atoms per compute tile (per partition)

_RUNNER_CACHE = {}


# ---------------------------------------------------------------------------
# walrus compat: this build rejects >1 sync-wait per instruction.  Split
# overflow waits onto NoOps inserted immediately before, same engine/block.
# ---------------------------------------------------------------------------
def _fix_bir_json(bir_json):
    m = _json.loads(bir_json)
    for fn in m.get("functions", []):
        for blk in fn.get("blocks", []):
            out = []
            for inst in blk.get("instructions", []):
                si = inst.get("sync_info")
                waits = (si or {}).get("on_wait", [])
                if len(waits) > 1:
                    for k, w in enumerate(waits[:-1]):
                        out.append({
                            "debug": inst.get("debug", 0),
                            "engine": inst["engine"],
                            "ins": [],
                            "name": f"{inst['name']}-sw{k}",
                            "opcode": "NoOp",
                            "outs": [],
                            "sync_info": {"on_update": [], "on_wait": [w]},
                        })
                    si["on_wait"] = [waits[-1]]
                out.append(inst)
            blk["instructions"] = out
    return _json.dumps(m).encode()


_PATCHED = False


def _install_compat():
    global _PATCHED
    if _PATCHED:
        return
    _PATCHED = True
    import concourse.bass_utils as bu
    import concourse.bass2jax as b2j
    orig = bu.compile_bir_kernel

    def patched(bir_json, tmpdir, neff_name="file.neff"):
        return orig(_fix_bir_json(bir_json), tmpdir, neff_name)

    bu.compile_bir_kernel = patched
    b2j.compile_bir_kernel = patched


# ---------------------------------------------------------------------------
# device program
# ---------------------------------------------------------------------------
def _build_nc(K):
    import concourse.bass as bass
    import concourse.mybir as mybir
    import concourse.tile as tile

    F = CAT * K
    nc = bass.Bass()
    d_in = nc.declare_dram_parameter("d", [P, APP * K], mybir.dt.float32, isOutput=False)
    q_in = nc.declare_dram_parameter("qj", [P, APP * K], mybir.dt.float32, isOutput=False)
    qic_in = nc.declare_dram_parameter("qic", [P, APP], mybir.dt.float32, isOutput=False)
    e_out = nc.declare_dram_parameter("E", [P, APP], mybir.dt.float32, isOutput=True)

    AL = mybir.AluOpType
    AF = mybir.ActivationFunctionType

    with tile.TileContext(nc, num_cores=N_CORES) as tc:
        with tc.tile_pool(name="io", bufs=3) as io, \
             tc.tile_pool(name="tmp", bufs=1) as tp, \
             tc.tile_pool(name="accp", bufs=1) as ap_pool:
            acc = ap_pool.tile([P, APP], mybir.dt.float32)
            qic = ap_pool.tile([P, APP], mybir.dt.float32)
            nc.sync.dma_start(qic[:], qic_in[:])
            n_chunks = APP // CAT
            for c in range(n_chunks):
                sl = slice(c * CAT * K, (c + 1) * CAT * K)
                D = io.tile([P, F], mybir.dt.float32, tag="D")
                Q = io.tile([P, F], mybir.dt.float32, tag="Q")
                nc.sync.dma_start(D[:], d_in[:, sl])
                nc.sync.dma_start(Q[:], q_in[:, sl])

                x2 = tp.tile([P, F], mybir.dt.float32, tag="x2")
                phi = tp.tile([P, F], mybir.dt.float32, tag="phi")
                rcp = tp.tile([P, F], mybir.dt.float32, tag="rcp")
                x = tp.tile([P, F], mybir.dt.float32, tag="x")
                t = tp.tile([P, F], mybir.dt.float32, tag="t")
                x3 = tp.tile([P, F], mybir.dt.float32, tag="x3")
                inr = tp.tile([P, F], mybir.dt.float32, tag="inr")
                r = tp.tile([P, F], mybir.dt.float32, tag="r")
                term = tp.tile([P, F], mybir.dt.float32, tag="term")

                # phi = 1/sqrt(d^2+1), rcp = 1/d
                nc.vector.tensor_tensor(x2[:], D[:], D[:], op=AL.mult)
                nc.scalar.activation(x2[:], x2[:], AF.Sqrt, bias=1.0)
                nc.vector.reciprocal(phi[:], x2[:])
                nc.vector.reciprocal(rcp[:], D[:])
                # f(2d) with x = clamp(2d/CUTOFF, <=1):  f-1 = x^3*(x*(15-6x)-10)
                nc.vector.tensor_scalar(x[:], D[:], 2.0 / CUTOFF, 1.0,
                                        op0=AL.mult, op1=AL.min)
                nc.vector.tensor_tensor(t[:], x[:], x[:], op=AL.mult)
                nc.vector.tensor_tensor(x3[:], t[:], x[:], op=AL.mult)
                nc.vector.tensor_scalar(inr[:], x[:], -6.0, 15.0,
                                        op0=AL.mult, op1=AL.add)
                nc.vector.tensor_tensor(inr[:], x[:], inr[:], op=AL.mult)
                nc.vector.tensor_scalar(inr[:], inr[:], -10.0, None, op0=AL.add)
                nc.vector.tensor_tensor(r[:], x3[:], inr[:], op=AL.mult)
                # chi = phi + (f-1)*(phi - rcp)
                nc.vector.tensor_tensor(x2[:], phi[:], rcp[:], op=AL.subtract)
                nc.vector.tensor_tensor(r[:], r[:], x2[:], op=AL.mult)
                nc.vector.tensor_tensor(r[:], r[:], phi[:], op=AL.add)
                # term = qj_c * chi ; per-atom sum over K slots
                nc.vector.tensor_tensor(term[:], r[:], Q[:], op=AL.mult)
                nc.vector.tensor_reduce(
                    acc[:, c * CAT:(c + 1) * CAT],
                    term[:].rearrange("p (a k) -> p a k", k=K),
                    axis=mybir.AxisListType.X,
                    op=AL.add,
                )
            # E = acc * qi_c * 0.5
            nc.vector.tensor_tensor(acc[:], acc[:], qic[:], op=AL.mult)
            nc.vector.tensor_scalar(acc[:], acc[:], 0.5, None, op0=AL.mult)
            nc.sync.dma_start(e_out[:], acc[:])
    return nc


class _Runner:
    """Compile once; keep a reusable jitted SPMD callable."""

    def __init__(self, nc):
        import jax
        from jax.sharding import Mesh, PartitionSpec, NamedSharding
        from jax.experimental.shard_map import shard_map
        import concourse.mybir as mybir
        import concourse.bass2jax as b2j
        b2j.install_neuronx_cc_hook()
        self.jax = jax
        in_names, out_names, out_avals, zero_outs = [], [], [], []
        pname = nc.partition_id_tensor.name if nc.partition_id_tensor else None
        for alloc in nc.m.functions[0].allocations:
            if not isinstance(alloc, mybir.MemoryLocationSet):
                continue
            name = alloc.memorylocations[0].name
            if alloc.kind == "ExternalInput":
                if name != pname:
                    in_names.append(name)
            elif alloc.kind == "ExternalOutput":
                shape = tuple(alloc.tensor_shape)
                dtype = mybir.dt.np(alloc.dtype)
                out_names.append(name)
                out_avals.append(jax.core.ShapedArray(shape, dtype))
                zero_outs.append(np.zeros(shape, dtype))
        self.in_names, self.out_names = in_names, out_names
        self.out_avals, self.zero_outs = out_avals, zero_outs
        all_in = list(in_names) + list(out_names) + ([pname] if pname else [])

        def _body(*args):
            operands = list(args)
            if pname is not None:
                operands.append(b2j.partition_id_tensor())
            return tuple(b2j._bass_exec_p.bind(
                *operands,
                out_avals=tuple(out_avals),
                in_names=tuple(all_in),
                out_names=tuple(out_names),
                lowering_input_output_aliases=(),
                sim_require_finite=True,
                sim_require_nnan=True,
                nc=nc,
            ))

        devices = jax.devices()[:N_CORES]
        mesh = Mesh(np.asarray(devices), ("core",))
        n_in = len(in_names) + len(zero_outs)
        self.fn = jax.jit(
            shard_map(_body, mesh=mesh,
                      in_specs=(PartitionSpec("core"),) * n_in,
                      out_specs=(PartitionSpec("core"),) * len(out_names),
                      check_rep=False),
            keep_unused=True,
        )
        self.sharding = NamedSharding(mesh, PartitionSpec("core"))

    def put_inputs(self, in_maps, device_resident=False):
        args = []
        for name in self.in_names:
            cat = np.concatenate([np.asarray(m[name]) for m in in_maps], axis=0)
            args.append(cat)
        for z in self.zero_outs:
            args.append(np.zeros((N_CORES * z.shape[0], *z.shape[1:]), z.dtype))
        if device_resident:
            # keeps repeat-timing free of host->device transfer; the reshard
            # program can fail to compile on some stacks, so fall back to np.
            try:
                args = [self.jax.device_put(a, self.sharding) for a in args]
                self.jax.block_until_ready(args)
            except Exception:
                pass
        return args

    def run(self, args):
        outs = self.fn(*args)
        self.jax.block_until_ready(outs)
        return outs

    def results(self, outs):
        res = []
        for c in range(N_CORES):
            res.append({
                name: np.asarray(outs[i]).reshape(N_CORES, *self.out_avals[i].shape)[c]
                for i, name in enumerate(self.out_names)
            })
        return res


def _get_runner(K):
    if K not in _RUNNER_CACHE:
        _install_compat()
        _RUNNER_CACHE[K] = _Runner(_build_nc(K))
    return _RUNNER_CACHE[K]


# ---------------------------------------------------------------------------
# host-side shard construction
# ---------------------------------------------------------------------------
def _prep(qi, edge_dist, edge_index, q_ref, N, atom_mol_batch):
    qi = np.asarray(qi, np.float32)
    edge_dist = np.asarray(edge_dist, np.float32)
    ii = np.asarray(edge_index[0], np.int64)
    jj = np.asarray(edge_index[1], np.int64)
    # charge-neutrality correction (index-driven segment sum over atoms)
    q_mol = np.bincount(np.asarray(atom_mol_batch, np.int64), weights=qi,
                        minlength=N_MOL).astype(np.float32)
    corr = (q_mol - np.asarray(q_ref, np.float32)) / np.asarray(N, np.float32)
    qi_c = qi - corr[np.asarray(atom_mol_batch, np.int64)]
    qj_c = qi_c[jj]

    # CSR by target atom with fixed K slots per atom
    order = np.argsort(ii, kind="stable")
    i_s = ii[order]
    counts = np.bincount(ii, minlength=N_ATOMS)
    K = int(counts.max())
    K = ((K + 3) // 4) * 4
    offs = np.zeros(N_ATOMS, np.int64)
    np.cumsum(counts[:-1], out=offs[1:])
    slot = np.arange(N_EDGES, dtype=np.int64) - offs[i_s]
    pos = i_s * K + slot
    dpad = np.ones(N_ATOMS * K, np.float32)
    qpad = np.zeros(N_ATOMS * K, np.float32)
    dpad[pos] = edge_dist[order]
    qpad[pos] = qj_c[order]
    return qi_c, dpad, qpad, K


def kernel(qi, edge_dist, edge_index, q_ref, N, atom_mol_batch):
    qi_c, dpad, qpad, K = _prep(qi, edge_dist, edge_index, q_ref, N,
                                atom_mol_batch)
    runner = _get_runner(K)
    apc = N_ATOMS // N_CORES
    in_maps = []
    for c in range(N_CORES):
        a0 = c * apc
        in_maps.append({
            "d": dpad[a0 * K:(a0 + apc) * K].reshape(P, APP * K),
            "qj": qpad[a0 * K:(a0 + apc) * K].reshape(P, APP * K),
            "qic": qi_c[a0:a0 + apc].reshape(P, APP),
        })
    args = runner.put_inputs(in_maps)
    res = runner.results(runner.run(args))
    out = np.concatenate([r["E"].reshape(apc) for r in res])
    return out.astype(np.float32)
